# revision 7
# baseline (speedup 1.0000x reference)
"""MeshConv (GNN message passing) Bass kernel for 8 trn2 NeuronCores.

Strategy (v4: SBUF-resident table + GPSIMD ap_gather, no SWDGE)
---------------------------------------------------------------
Shard (batch, edge-half): core c handles batch c//2, edge half c%2
(EH=100000 edges each).

The whole per-batch feature table lives in SBUF in a pair-packed layout:
partition 16*w + q holds features (2q, 2q+1) of edge window w (NW=8
windows of WE=25088 edges), bf16 -> 100KB/partition.  A gather of one
token (32 feats) is then a 4-byte read on each of 16 partitions, which
is exactly what the ap_gather Q7 ucode does - and each of the 8 Q7
cores (16-partition groups) runs an independent index list, so the 8
window buckets gather in parallel inside ONE instruction.

Per CE=1792-edge chunk (5*CE = 8960 tokens):
  g1   one ap_gather: group w gathers the chunk's window-w bucket
       (host-bucketed, cap-padded) from its table slab -> OC compact.
  repl 8 fold DMAs (OC slab w -> CT slab 7 block w) + 3 log-tree
       broadcast DMAs replicate the compact tile CT to all 8 slabs.
  g2   one ap_gather: group g un-permutes its edge slice's 5 tap lists
       [f1|f3|f2|f4|f0] (SL=224 edges each) from CT -> G slot-major.
  DVE/ACT combine: T1=f1+f3, T2=f2+f4, D1=|f1-f3|, D2=|f2-f4|, F0.
  fold 8 DMAs unfold pair layout [16,n,2] -> [32,n] per group -> GM.
  5 accumulating K=32 matmuls per group into PSUM, bias on DVE during
  PSUM->SBUF, result DMAed out as f32.
"""

import sys

sys.path.insert(0, "/opt/trn_rl_repo")

import numpy as np
import ml_dtypes

B, C_IN, C_OUT, E, KK = 4, 32, 64, 200000, 5
NCORES = 8

NW = 8  # windows = Q7 core groups
WE = 25088  # edges per window (8*25088 = 200704 >= E)
CE = 1792  # edges per chunk
SL = CE // NW  # edges per un-permute group slice
NI2 = KK * SL  # un-permute indices per group
EH = E // 2  # edges per core
NCH = -(-EH // CE)  # chunks per core
EPAD = NCH * CE
TAP_ORDER = (1, 3, 2, 4, 0)  # [f1|f3|f2|f4|f0] per group

_PROG_CACHE = {}


def _build(cap, reps=1, passthrough=False):
    key = (cap, reps, passthrough)
    if key in _PROG_CACHE:
        return _PROG_CACHE[key]
    import concourse.bass as bass
    import concourse.bacc as bacc
    import concourse.tile as tile
    from concourse import mybir

    TS = NW * cap  # compact tile tokens
    dt = mybir.dt
    AT = mybir.AluOpType
    nc = bacc.Bacc("TRN2", target_bir_lowering=False, debug=False)
    table_d = nc.dram_tensor("table", [128, WE * 2], dt.bfloat16, kind="ExternalInput")
    g1_d = nc.dram_tensor("g1idx", [NCH, 128, cap // 16], dt.int16, kind="ExternalInput")
    g2_d = nc.dram_tensor("g2idx", [NCH, 128, NI2 // 16], dt.int16, kind="ExternalInput")
    wts_d = nc.dram_tensor("wts", [C_IN, KK * C_OUT], dt.bfloat16, kind="ExternalInput")
    bias_d = nc.dram_tensor("bias", [C_OUT, 1], dt.float32, kind="ExternalInput")
    out_d = nc.dram_tensor("out", [C_OUT, EPAD], dt.float32, kind="ExternalOutput")

    if passthrough:
        with tile.TileContext(nc) as tc:
            with tc.tile_pool(name="pt", bufs=1) as ptp:
                z = ptp.tile([C_OUT, CE], dt.float32)
                nc.vector.memset(z[:], 0.0)
                for ch in range(NCH):
                    nc.sync.dma_start(out_d[:, ch * CE : (ch + 1) * CE], z[:])
        nc.compile()
        _PROG_CACHE[key] = nc
        return nc

    with tile.TileContext(nc) as tc:
        with (
            tc.tile_pool(name="const", bufs=1) as cp,
            tc.tile_pool(name="idxp", bufs=2) as ip,
            tc.tile_pool(name="ocp", bufs=2) as ocp,
            tc.tile_pool(name="ctp", bufs=1) as ctp,
            tc.tile_pool(name="gp", bufs=2) as gpp,
            tc.tile_pool(name="cmp", bufs=1) as cmp_,
            tc.tile_pool(name="gmp", bufs=1) as gmp,
            tc.tile_pool(name="obp", bufs=2) as obp,
            tc.tile_pool(name="psp", bufs=8, space="PSUM") as pp,
        ):
            table = cp.tile([128, WE * 2], dt.bfloat16)
            nc.sync.dma_start(table[:], table_d[:])
            wts = cp.tile([C_IN, KK * C_OUT], dt.bfloat16)
            nc.sync.dma_start(wts[:], wts_d[:])
            bias = cp.tile([C_OUT, 1], dt.float32)
            nc.sync.dma_start(bias[:], bias_d[:])
            # CT single-buffered: fold(ch) waits g2(ch-1) via WAR, hidden
            # under g1(ch+1) on the Q7 engine.
            ct = ctp.tile([128, TS * 2], dt.bfloat16)

            for _ in range(reps):
                state = {}

                def emit_g1(ch):
                    it1 = ip.tile([128, cap // 16], dt.int16, tag="it1")
                    nc.sync.dma_start(it1[:], g1_d[ch])
                    oc = ocp.tile([128, cap * 2], dt.bfloat16, tag="oc")
                    nc.gpsimd.ap_gather(oc[:], table[:], it1[:], 128, WE, 2, cap)
                    state[ch] = oc

                def emit_rest(ch):
                    oc = state.pop(ch)
                    # fold: OC slab w -> CT slab 7, block w
                    for w in range(NW):
                        nc.sync.dma_start(
                            ct[112:128, w * cap * 2 : (w + 1) * cap * 2],
                            oc[16 * w : 16 * w + 16, :],
                        )
                    # log-tree broadcast to slabs 0..6
                    nc.sync.dma_start(ct[96:112, :], ct[112:128, :])
                    nc.sync.dma_start(ct[64:96, :], ct[96:128, :])
                    nc.sync.dma_start(ct[0:64, :], ct[64:128, :])

                    it2 = ip.tile([128, NI2 // 16], dt.int16, tag="it2")
                    nc.sync.dma_start(it2[:], g2_d[ch])
                    g = gpp.tile([128, NI2 * 2], dt.bfloat16, tag="g")
                    nc.gpsimd.ap_gather(g[:], ct[:], it2[:], 128, TS, 2, NI2)

                    # combine: per group [f1|f3|f2|f4|f0] blocks of SL*2 elems
                    S = SL * 2
                    cm = cmp_.tile([128, KK * S], dt.bfloat16, tag="cm")
                    nc.vector.tensor_tensor(
                        out=cm[:, 0:S], in0=g[:, 0:S], in1=g[:, S : 2 * S], op=AT.add
                    )
                    nc.vector.tensor_tensor(
                        out=cm[:, S : 2 * S],
                        in0=g[:, 2 * S : 3 * S],
                        in1=g[:, 3 * S : 4 * S],
                        op=AT.add,
                    )
                    sc = cmp_.tile([128, 2 * S], dt.bfloat16, tag="sc")
                    nc.vector.tensor_tensor(
                        out=sc[:, 0:S], in0=g[:, 0:S], in1=g[:, S : 2 * S], op=AT.subtract
                    )
                    nc.vector.tensor_tensor(
                        out=sc[:, S : 2 * S],
                        in0=g[:, 2 * S : 3 * S],
                        in1=g[:, 3 * S : 4 * S],
                        op=AT.subtract,
                    )
                    nc.scalar.activation(
                        cm[:, 2 * S : 4 * S],
                        sc[:],
                        mybir.ActivationFunctionType.Abs,
                    )
                    nc.scalar.copy(cm[:, 4 * S : 5 * S], g[:, 4 * S : 5 * S])

                    # unfold pair layout: CM group g [16, (5*SL, 2)] -> GM
                    # [32, g, 5*SL] with partition p<16 = feature 2p (r=0),
                    # p>=16 = feature 2(p-16)+1 (weights rows permuted to match)
                    gm = gmp.tile([32, NW, KK * SL], dt.bfloat16, tag="gm")
                    for grp in range(NW):
                        src = cm[16 * grp : 16 * grp + 16, :].rearrange(
                            "p (n r) -> p r n", r=2
                        )
                        for r in range(2):
                            nc.sync.dma_start(
                                gm[16 * r : 16 * r + 16, grp, :], src[:, r : r + 1, :]
                            )

                    ob = obp.tile([C_OUT, CE], dt.float32, tag="ob")
                    for grp in range(NW):
                        ps = pp.tile([C_OUT, SL], dt.float32, tag="ps")
                        for k in range(KK):
                            nc.tensor.matmul(
                                ps[:],
                                wts[:, k * C_OUT : (k + 1) * C_OUT],
                                gm[:, grp, k * SL : (k + 1) * SL],
                                start=(k == 0),
                                stop=(k == KK - 1),
                            )
                        nc.vector.tensor_scalar(
                            out=ob[:, grp * SL : (grp + 1) * SL],
                            in0=ps[:],
                            scalar1=bias[:],
                            scalar2=None,
                            op0=AT.add,
                        )
                    nc.sync.dma_start(out_d[:, ch * CE : (ch + 1) * CE], ob[:])

                emit_g1(0)
                if NCH > 1:
                    emit_g1(1)
                for ch in range(NCH):
                    emit_rest(ch)
                    if ch + 2 < NCH:
                        emit_g1(ch + 2)
    nc.compile()
    _PROG_CACHE[key] = nc
    return nc


def _wrap16_groups(a):
    """(NCH, 8, N) per-group lists -> (NCH, 128, N//16) idx tiles: group g's
    value i goes to partition 16g + i%16, column i//16."""
    nch, ng, n = a.shape
    b = a.reshape(nch, ng, n // 16, 16).transpose(0, 1, 3, 2)  # (NCH, 8, 16, N//16)
    return np.ascontiguousarray(b.reshape(nch, 128, n // 16))


def _marshal_core(gi):
    """gi: (EH, KK) int64 token ids in [0, E). Returns g1v (NCH, 8, cap-less
    bucket lists as dict), ranks etc. Done in two passes: first compute
    per-(chunk,window) counts to get cap, then fill."""
    gp = np.empty((EPAD, KK), np.int64)
    gp[:EH] = gi
    # pad edges: spread dummy tokens across windows so no bucket inflates cap
    npad = EPAD - EH
    if npad:
        gp[EH:] = (
            (np.arange(npad)[:, None] * KK + np.arange(KK)) % NW
        ) * WE
    # slot order: (chunk, group, tap in TAP_ORDER, edge in slice)
    t = gp.reshape(NCH, NW, SL, KK)[:, :, :, TAP_ORDER]  # (NCH, 8, SL, 5)
    tokS = t.transpose(0, 1, 3, 2).reshape(NCH, NW * NI2)  # group-major slots
    w = tokS // WE
    off = (tokS - w * WE).astype(np.int16)
    return tokS, w, off


def _counts_core(gi):
    tokS, w, off = _marshal_core(gi)
    cnt = np.zeros((NCH, NW), np.int64)
    for ch in range(NCH):
        cnt[ch] = np.bincount(w[ch], minlength=NW)
    return cnt


def _fill_core(gi, cap):
    tokS, w, off = _marshal_core(gi)
    g1v = np.zeros((NCH, NW, cap), np.int16)
    g2v = np.empty((NCH, NW * NI2), np.int16)
    for ch in range(NCH):
        wc, oc = w[ch], off[ch]
        order = np.argsort(wc, kind="stable")
        counts = np.bincount(wc, minlength=NW)
        starts = np.concatenate([[0], np.cumsum(counts)[:-1]])
        rank = np.empty(NW * NI2, np.int64)
        rank[order] = np.arange(NW * NI2) - starts[wc[order]]
        g1v[ch][wc, rank] = oc
        g2v[ch] = (wc * cap + rank).astype(np.int16)
    return (
        _wrap16_groups(g1v),
        _wrap16_groups(g2v.reshape(NCH, NW, NI2)),
    )


def _marshal_weights(W, b):
    Wk = np.asarray(W)[:, :, 0, :]  # (C_OUT, C_IN, 5)
    # CM tap blocks: [T1, T2, D1, D2, F0] -> weights [W1, W2, W3, W4, W0]
    worder = (1, 2, 3, 4, 0)
    wts = np.zeros((C_IN, KK * C_OUT), np.float32)
    for i, k in enumerate(worder):
        wts[:, i * C_OUT : (i + 1) * C_OUT] = Wk[:, :, k].T
    # GM partition p<16 = feature 2p, p>=16 = feature 2(p-16)+1
    perm = np.concatenate([np.arange(0, C_IN, 2), np.arange(1, C_IN, 2)])
    wts = wts[perm]
    bias = np.asarray(b).reshape(C_OUT, 1).astype(np.float32)
    return wts.astype(ml_dtypes.bfloat16), bias


def _marshal_table(xb):
    """xb: (C_IN, E) f32 -> [128, WE*2] bf16, partition 16w+q holds
    features (2q, 2q+1) of window w."""
    xp = np.zeros((C_IN, NW * WE), np.float32)
    xp[:, :E] = xb
    # (c, w, e) -> partition 16w + c//2, elem e*2 + c%2
    t = xp.reshape(16, 2, NW, WE)  # (q, r, w, e)
    t = t.transpose(2, 0, 3, 1)  # (w, q, e, r)
    return np.ascontiguousarray(t.reshape(128, WE * 2).astype(ml_dtypes.bfloat16))


def _run(x, Gi, W, b, reps=1, passthrough=False):
    from concourse.bass_utils import run_bass_kernel_spmd

    x = np.asarray(x)
    Gi = np.asarray(Gi)

    gi = {}
    for c in range(NCORES):
        bb, h = divmod(c, 2)
        gi[c] = Gi[bb, h * EH : (h + 1) * EH].astype(np.int64)

    cnts = np.stack([_counts_core(gi[c]) for c in range(NCORES)])
    cap = int(-(-int(cnts.max()) // 16) * 16)

    nc = _build(cap, reps, passthrough)
    wts, bias = _marshal_weights(W, b)
    tables = {bb: _marshal_table(x[bb]) for bb in range(B)}

    in_maps = []
    for c in range(NCORES):
        bb, h = divmod(c, 2)
        g1, g2 = _fill_core(gi[c], cap)
        in_maps.append(
            {"table": tables[bb], "g1idx": g1, "g2idx": g2, "wts": wts, "bias": bias}
        )
    res = run_bass_kernel_spmd(nc, in_maps, core_ids=list(range(NCORES)))
    out = np.empty((B, C_OUT, E, 1), np.float32)
    for c in range(NCORES):
        bb, h = divmod(c, 2)
        out[bb, :, h * EH : (h + 1) * EH, 0] = res.results[c]["out"][:, :EH]
    return out


def kernel(x, Gi, W, b):
    return _run(x, Gi, W, b)


# revision 24
# speedup vs baseline: 32.2438x; 32.2438x over previous
"""MeshConv (GNN message passing) Bass kernel for 8 trn2 NeuronCores.

Strategy (v4: SBUF-resident table + GPSIMD ap_gather, no SWDGE)
---------------------------------------------------------------
Shard (batch, edge-half): core c handles batch c//2, edge half c%2
(EH=100000 edges each).

The whole per-batch feature table lives in SBUF in a pair-packed layout:
partition 16*w + q holds features (2q, 2q+1) of edge window w (NW=8
windows of WE=25088 edges), bf16 -> 100KB/partition.  A gather of one
token (32 feats) is then a 4-byte read on each of 16 partitions, which
is exactly what the ap_gather Q7 ucode does - and each of the 8 Q7
cores (16-partition groups) runs an independent index list, so the 8
window buckets gather in parallel inside ONE instruction.

Per CE=1792-edge chunk (5*CE = 8960 tokens):
  g1   one ap_gather: group w gathers the chunk's window-w bucket
       (host-bucketed, cap-padded) from its table slab -> OC compact.
  repl 8 fold DMAs (OC slab w -> CT slab 7 block w) + 3 log-tree
       broadcast DMAs replicate the compact tile CT to all 8 slabs.
  g2   one ap_gather: group g un-permutes its edge slice's 5 tap lists
       [f1|f3|f2|f4|f0] (SL=224 edges each) from CT -> G slot-major.
  DVE/ACT combine: T1=f1+f3, T2=f2+f4, D1=|f1-f3|, D2=|f2-f4|, F0.
  fold 8 DMAs unfold pair layout [16,n,2] -> [32,n] per group -> GM.
  5 accumulating K=32 matmuls per group into PSUM, bias on DVE during
  PSUM->SBUF, result DMAed out as f32.
"""

import sys

sys.path.insert(0, "/opt/trn_rl_repo")

import numpy as np
import ml_dtypes

B, C_IN, C_OUT, E, KK = 4, 32, 64, 200000, 5
NCORES = 8

NW = 8  # windows = Q7 core groups
WE = 25088  # edges per window (8*25088 = 200704 >= E)
CE = 1792  # edges per chunk
SL = CE // NW  # edges per un-permute group slice
NI2 = KK * SL  # un-permute indices per group
EH = E // 2  # edges per core
NCH = -(-EH // CE)  # chunks per core
EPAD = NCH * CE
TAP_ORDER = (1, 3, 2, 4, 0)  # [f1|f3|f2|f4|f0] per group

_PROG_CACHE = {}


def _build(cap, reps=1, passthrough=False):
    import os

    ablate = os.environ.get("KABLATE", "")  # ""|g1only|noct|nog2|nomm
    key = (cap, reps, passthrough, ablate)
    if key in _PROG_CACHE:
        return _PROG_CACHE[key]
    import concourse.bass as bass
    import concourse.bacc as bacc
    import concourse.tile as tile
    from concourse import mybir

    TS = NW * cap  # compact tile tokens
    dt = mybir.dt
    AT = mybir.AluOpType
    nc = bacc.Bacc("TRN2", target_bir_lowering=False, debug=False)
    table_d = nc.dram_tensor("table", [128, WE * 2], dt.bfloat16, kind="ExternalInput")
    g1_d = nc.dram_tensor("g1idx", [NCH, 128, cap // 16], dt.int16, kind="ExternalInput")
    g2_d = nc.dram_tensor("g2idx", [NCH, 128, NI2 // 16], dt.int16, kind="ExternalInput")
    wts_d = nc.dram_tensor("wts", [C_IN, KK * C_OUT], dt.bfloat16, kind="ExternalInput")
    bias_d = nc.dram_tensor("bias", [C_OUT, 1], dt.float32, kind="ExternalInput")
    out_d = nc.dram_tensor("out", [C_OUT, EPAD], dt.bfloat16, kind="ExternalOutput")

    if passthrough:
        with tile.TileContext(nc) as tc:
            with tc.tile_pool(name="pt", bufs=1) as ptp:
                z = ptp.tile([C_OUT, CE], dt.bfloat16)
                nc.vector.memset(z[:], 0.0)
                for ch in range(NCH):
                    nc.sync.dma_start(out_d[:, ch * CE : (ch + 1) * CE], z[:])
        nc.compile()
        _PROG_CACHE[key] = nc
        return nc

    with tile.TileContext(nc) as tc:
        with (
            tc.tile_pool(name="const", bufs=1) as cp,
            tc.tile_pool(name="idxp", bufs=2) as ip,
            tc.tile_pool(name="ocp", bufs=2) as ocp,
            tc.tile_pool(name="ctp", bufs=1) as ctp,
            tc.tile_pool(name="gp", bufs=2) as gpp,
            tc.tile_pool(name="cmp", bufs=1) as cmp_,
            tc.tile_pool(name="gmp", bufs=1) as gmp,
            tc.tile_pool(name="obp", bufs=2) as obp,
            tc.tile_pool(name="psp", bufs=4, space="PSUM") as pp,
        ):
            table = cp.tile([128, WE * 2], dt.bfloat16)
            nc.sync.dma_start(table[:], table_d[:])
            wts = cp.tile([C_IN, KK * C_OUT], dt.bfloat16)
            nc.sync.dma_start(wts[:], wts_d[:])
            bias = cp.tile([C_OUT, 1], dt.float32)
            nc.sync.dma_start(bias[:], bias_d[:])
            # CT single-buffered: fold(ch) waits g2(ch-1) via WAR, hidden
            # under g1(ch+1) on the Q7 engine.
            ct = ctp.tile([128, TS * 2], dt.bfloat16)
            if ablate == "noct":
                nc.vector.memset(ct[:, 0:128], 0.0)

            import contextlib

            rep_ctx = tc.For_i(0, reps) if reps > 1 else contextlib.nullcontext()
            with rep_ctx:
                state = {}

                def emit_g1(ch):
                    it1 = ip.tile([128, cap // 16], dt.int16, tag="it1")
                    nc.sync.dma_start(it1[:], g1_d[ch])
                    it2 = ip.tile([128, NI2 // 16], dt.int16, tag="it2")
                    nc.sync.dma_start(it2[:], g2_d[ch])
                    oc = ocp.tile([128, cap * 2], dt.bfloat16, tag="oc")
                    nc.gpsimd.ap_gather(oc[:], table[:], it1[:], 128, WE, 2, cap)
                    state[ch] = (oc, it2)

                def emit_rest(ch):
                    oc, it2 = state.pop(ch)
                    if ablate == "g1only":
                        return
                    if ablate != "noct":
                        # fold: OC slab w -> CT slab 7, block w
                        for w in range(NW):
                            nc.sync.dma_start(
                                ct[112:128, w * cap * 2 : (w + 1) * cap * 2],
                                oc[16 * w : 16 * w + 16, :],
                            )
                        # log-tree broadcast to slabs 0..6
                        nc.sync.dma_start(ct[96:112, :], ct[112:128, :])
                        nc.sync.dma_start(ct[64:96, :], ct[96:128, :])
                        nc.sync.dma_start(ct[0:64, :], ct[64:128, :])

                    g = gpp.tile([128, NI2 * 2], dt.bfloat16, tag="g")
                    if ablate == "nog2":
                        nc.vector.memset(g[:, 0:128], 0.0)
                    else:
                        nc.gpsimd.ap_gather(g[:], ct[:], it2[:], 128, TS, 2, NI2)

                    # combine into parity planes: CMP[:, r, :] = [T1|T2|D1|D2|F0]
                    # (strided reads of G, contiguous writes, so the unfold
                    # below is plain rectangular DMAs); 3D APs do both
                    # parities in one op.
                    g3 = g[:].rearrange("p (j i r) -> p j r i", j=KK, i=SL, r=2)

                    def gs(j):
                        return g3[:, j : j + 1, :, :].squeeze(axis=1)

                    cm = cmp_.tile([128, 2, KK * SL], dt.bfloat16, tag="cm")
                    sc = cmp_.tile([128, 2, 2 * SL], dt.bfloat16, tag="sc")
                    nc.vector.tensor_tensor(
                        out=cm[:, :, 0:SL], in0=gs(0), in1=gs(1), op=AT.add
                    )
                    nc.vector.tensor_tensor(
                        out=cm[:, :, SL : 2 * SL], in0=gs(2), in1=gs(3), op=AT.add
                    )
                    nc.vector.tensor_tensor(
                        out=sc[:, :, 0:SL], in0=gs(0), in1=gs(1), op=AT.subtract
                    )
                    nc.vector.tensor_tensor(
                        out=sc[:, :, SL : 2 * SL], in0=gs(2), in1=gs(3), op=AT.subtract
                    )
                    # |x| on DVE: clear the bf16 sign bit via int16 bitcast
                    nc.vector.tensor_scalar(
                        out=cm[:, :, 2 * SL : 4 * SL].bitcast(dt.int16),
                        in0=sc[:].bitcast(dt.int16),
                        scalar1=0x7FFF,
                        scalar2=None,
                        op0=AT.bitwise_and,
                    )
                    nc.vector.tensor_scalar(
                        out=cm[:, :, 4 * SL : 5 * SL],
                        in0=gs(4),
                        scalar1=0.0,
                        scalar2=None,
                        op0=AT.add,
                    )

                    if ablate == "nomm":
                        return
                    # unfold: GM2 [32, tap, CE] with partition p<16 = feature
                    # 2p (r=0), p>=16 = feature 2(p-16)+1 (weights rows
                    # permuted to match); DMAs spread over ACT/PE queues
                    gm = gmp.tile([32, KK, CE], dt.bfloat16, tag="gm")
                    for grp in range(NW):
                        for r in range(2):
                            eng = nc.scalar
                            eng.dma_start(
                                gm[16 * r : 16 * r + 16, :, grp * SL : (grp + 1) * SL],
                                cm[16 * grp : 16 * grp + 16, r, :],
                            )

                    PB = CE // 4
                    ob = obp.tile([C_OUT, CE], dt.bfloat16, tag="ob")
                    for pb in range(4):
                        ps = pp.tile([C_OUT, PB], dt.float32, tag="ps")
                        for k in range(KK):
                            nc.tensor.matmul(
                                ps[:],
                                wts[:, k * C_OUT : (k + 1) * C_OUT],
                                gm[:, k, pb * PB : (pb + 1) * PB],
                                start=(k == 0),
                                stop=(k == KK - 1),
                            )
                        nc.vector.tensor_scalar(
                            out=ob[:, pb * PB : (pb + 1) * PB],
                            in0=ps[:],
                            scalar1=bias[:],
                            scalar2=None,
                            op0=AT.add,
                        )
                    nc.sync.dma_start(out_d[:, ch * CE : (ch + 1) * CE], ob[:])

                emit_g1(0)
                if NCH > 1:
                    emit_g1(1)
                for ch in range(NCH):
                    emit_rest(ch)
                    if ch + 2 < NCH:
                        emit_g1(ch + 2)
    nc.compile()
    _PROG_CACHE[key] = nc
    return nc


def _wrap16_groups(a):
    """(NCH, 8, N) per-group lists -> (NCH, 128, N//16) idx tiles: group g's
    value i goes to partition 16g + i%16, column i//16."""
    nch, ng, n = a.shape
    b = a.reshape(nch, ng, n // 16, 16).transpose(0, 1, 3, 2)  # (NCH, 8, 16, N//16)
    return np.ascontiguousarray(b.reshape(nch, 128, n // 16))


def _marshal_core(gi):
    """gi: (EH, KK) int64 token ids in [0, E). Returns g1v (NCH, 8, cap-less
    bucket lists as dict), ranks etc. Done in two passes: first compute
    per-(chunk,window) counts to get cap, then fill."""
    gp = np.empty((EPAD, KK), np.int64)
    gp[:EH] = gi
    # pad edges: spread dummy tokens across windows so no bucket inflates cap
    npad = EPAD - EH
    if npad:
        gp[EH:] = (
            (np.arange(npad)[:, None] * KK + np.arange(KK)) % NW
        ) * WE
    # slot order: (chunk, group, tap in TAP_ORDER, edge in slice)
    t = gp.reshape(NCH, NW, SL, KK)[:, :, :, TAP_ORDER]  # (NCH, 8, SL, 5)
    tokS = t.transpose(0, 1, 3, 2).reshape(NCH, NW * NI2)  # group-major slots
    w = tokS // WE
    off = (tokS - w * WE).astype(np.int16)
    return tokS, w, off


def _counts_core(gi):
    tokS, w, off = _marshal_core(gi)
    cnt = np.zeros((NCH, NW), np.int64)
    for ch in range(NCH):
        cnt[ch] = np.bincount(w[ch], minlength=NW)
    return cnt


def _fill_core(gi, cap):
    tokS, w, off = _marshal_core(gi)
    g1v = np.zeros((NCH, NW, cap), np.int16)
    g2v = np.empty((NCH, NW * NI2), np.int16)
    for ch in range(NCH):
        wc, oc = w[ch], off[ch]
        # sort buckets by token value: g1 reads become ascending (the Q7
        # SIMD_RD path is ~100x slower for random reads spanning >64KB)
        order = np.argsort(tokS[ch], kind="stable")
        counts = np.bincount(wc, minlength=NW)
        starts = np.concatenate([[0], np.cumsum(counts)[:-1]])
        rank = np.empty(NW * NI2, np.int64)
        rank[order] = np.arange(NW * NI2) - starts[wc[order]]
        g1v[ch][wc, rank] = oc
        g2v[ch] = (wc * cap + rank).astype(np.int16)
    return (
        _wrap16_groups(g1v),
        _wrap16_groups(g2v.reshape(NCH, NW, NI2)),
    )


def _marshal_weights(W, b):
    Wk = np.asarray(W)[:, :, 0, :]  # (C_OUT, C_IN, 5)
    # CM tap blocks: [T1, T2, D1, D2, F0] -> weights [W1, W2, W3, W4, W0]
    worder = (1, 2, 3, 4, 0)
    wts = np.zeros((C_IN, KK * C_OUT), np.float32)
    for i, k in enumerate(worder):
        wts[:, i * C_OUT : (i + 1) * C_OUT] = Wk[:, :, k].T
    # GM partition p<16 = feature 2p, p>=16 = feature 2(p-16)+1
    perm = np.concatenate([np.arange(0, C_IN, 2), np.arange(1, C_IN, 2)])
    wts = wts[perm]
    bias = np.asarray(b).reshape(C_OUT, 1).astype(np.float32)
    return wts.astype(ml_dtypes.bfloat16), bias


def _marshal_table(xb):
    """xb: (C_IN, E) f32 -> [128, WE*2] bf16, partition 16w+q holds
    features (2q, 2q+1) of window w."""
    xp = np.zeros((C_IN, NW * WE), np.float32)
    xp[:, :E] = xb
    # (c, w, e) -> partition 16w + c//2, elem e*2 + c%2
    t = xp.reshape(16, 2, NW, WE)  # (q, r, w, e)
    t = t.transpose(2, 0, 3, 1)  # (w, q, e, r)
    return np.ascontiguousarray(t.reshape(128, WE * 2).astype(ml_dtypes.bfloat16))


def _run(x, Gi, W, b, reps=1, passthrough=False):
    from concourse.bass_utils import run_bass_kernel_spmd

    x = np.asarray(x)
    Gi = np.asarray(Gi)

    gi = {}
    for c in range(NCORES):
        bb, h = divmod(c, 2)
        gi[c] = Gi[bb, h * EH : (h + 1) * EH].astype(np.int64)

    cnts = np.stack([_counts_core(gi[c]) for c in range(NCORES)])
    cap = int(-(-int(cnts.max()) // 16) * 16)

    nc = _build(cap, reps, passthrough)
    wts, bias = _marshal_weights(W, b)
    tables = {bb: _marshal_table(x[bb]) for bb in range(B)}

    in_maps = []
    for c in range(NCORES):
        bb, h = divmod(c, 2)
        g1, g2 = _fill_core(gi[c], cap)
        in_maps.append(
            {"table": tables[bb], "g1idx": g1, "g2idx": g2, "wts": wts, "bias": bias}
        )
    res = run_bass_kernel_spmd(nc, in_maps, core_ids=list(range(NCORES)))
    out = np.empty((B, C_OUT, E, 1), np.float32)
    for c in range(NCORES):
        bb, h = divmod(c, 2)
        out[bb, :, h * EH : (h + 1) * EH, 0] = res.results[c]["out"][:, :EH].astype(
            np.float32
        )
    return out


def kernel(x, Gi, W, b):
    return _run(x, Gi, W, b)
